# revision 2
# baseline (speedup 1.0000x reference)
"""ConvHex (hex-grid graph conv) Trainium2 Bass kernel.

out[b,o,h] = (Wc@x[b,:,h] + sum_k Wn[:,:,k]@x[b,:,nb[h,k]]*mask) / (1+#valid) + bias

Strategy (8 NeuronCores, data-parallel over batch B=256 -> 32/core):
- x staged ONCE in SBUF as a token table xt [128, 9*2048] bf16: token t
  (= hex id, 1039 = zero pad) is the 4KB column x[0:32, 0:64, t] stored on
  partition t%128 at free offset (t//128)*4KB.
- Neighbor gather: SBUF-source dma_gather(transpose=True), 4KB tokens
  (one descriptor = one hex for all 32 batches). Off the HBM port: the
  22MB of gather traffic rides the 435GB/s SBUF fabric instead.
- h count-sorted (desc valid-neighbor count) so slot k is active only for
  the first nk[k] columns; gathers and matmul widths trimmed per slot.
- Matmul: center (start=True, full width) then slots narrow->wide, slot 0
  last (stop=True, full width). K=64 contraction, batch pairs: even batch
  on PE rows 0-63 -> psum_e, odd on 64-127 -> psum_o.
- Epilogue: DVE multiply by 1/(1+count) broadcast, bf16 out. bias added
  on host only if nonzero (zero in this problem).
"""
import os
import numpy as np
import ml_dtypes

B, C_IN, C_OUT, H, K = 256, 64, 128, 1039, 6
NCORES = 8
BL = B // NCORES            # 32 batches per core
NPAIR = BL // 2             # 16
Hp = H + 1                  # 1040; token/column H (=1039) is the zero pad
NTOK = 1152                 # 9 ranks x 128 tokens (tokens >= Hp are zero)
NRANK = NTOK // 128
TOKB = BL * C_IN * 2        # 4096 bytes per token
HCS = [384, 384, 272]       # h-chunks (psum bank sized)
HC_OFF = [0, 384, 768]
BF16 = ml_dtypes.bfloat16

TRACE = bool(int(os.environ.get("KERNEL_TRACE", "0")))
LAST_RESULT = None

_CACHE = {}


def _build_program(segs, totw):
    import concourse.mybir as mybir
    import concourse.tile as tile
    from concourse import bacc

    nc = bacc.Bacc(name="convhex")
    dt = mybir.dt
    xt_d = nc.dram_tensor("xt", [128, NRANK * BL * C_IN], dt.bfloat16,
                          kind="ExternalInput")
    xc_d = nc.dram_tensor("xc", [NPAIR, 128, Hp], dt.bfloat16,
                          kind="ExternalInput")
    wt_d = nc.dram_tensor("wt", [128, 7 * 128], dt.bfloat16,
                          kind="ExternalInput")
    inv_d = nc.dram_tensor("inv", [128, Hp], dt.float32, kind="ExternalInput")
    it_d = nc.dram_tensor("idxt", [128, totw], dt.int16, kind="ExternalInput")
    y = nc.dram_tensor("y", [BL, 128, H], dt.bfloat16, kind="ExternalOutput")

    by_chunk = [[s for s in segs if s[0] == c] for c in range(len(HCS))]

    with tile.TileContext(nc) as tc:
        with tc.tile_pool(name="const", bufs=1) as cpool, \
             tc.tile_pool(name="gat", bufs=9) as gpool, \
             tc.tile_pool(name="xcp", bufs=8) as xcpool, \
             tc.tile_pool(name="osb", bufs=2) as opool, \
             tc.tile_pool(name="ps", bufs=2, space="PSUM") as pspool:
            wtile = cpool.tile([128, 7 * 128], dt.bfloat16)
            nc.sync.dma_start(wtile[:], wt_d[:, :])
            invt = cpool.tile([128, Hp], dt.float32)
            nc.sync.dma_start(invt[:], inv_d[:, :])
            itt = cpool.tile([128, totw], dt.int16)
            nc.sync.dma_start(itt[:], it_d[:, :])
            xtt = cpool.tile([128, NRANK * BL * C_IN], dt.bfloat16)
            nc.sync.dma_start(xtt[:], xt_d[:, :])

            for hci, hn in enumerate(HCS):
                off = HC_OFF[hci]
                hv = min(hn, H - off)   # valid output columns
                gts = {}
                for (_, k, col, gkn, wk) in by_chunk[hci]:
                    gt = gpool.tile([128, NPAIR, gkn], dt.bfloat16,
                                    tag="g", name=f"g_{hci}_{k}")
                    nc.gpsimd.dma_gather(
                        gt[:], xtt[:],
                        itt[:, col:col + gkn // 16],
                        num_idxs=gkn, num_idxs_reg=gkn,
                        elem_size=BL * C_IN,
                        transpose=True,
                        sbuf_tokens_per_rank=128,
                        sbuf_free_dim_per_rank=TOKB,
                    )
                    gts[k] = gt
                for blk in range(NPAIR // 2):
                    ps = []
                    xs = []
                    for j in range(2):
                        p = 2 * blk + j
                        xct = xcpool.tile([128, 384], dt.bfloat16, tag="xc")
                        nc.sync.dma_start(xct[:, 0:hn], xc_d[p, :, off:off + hn])
                        xs.append(xct)
                        pse = pspool.tile([128, 384], dt.float32, tag=f"pe{j}",
                                          name=f"pse_{hci}_{blk}_{j}")
                        pso = pspool.tile([128, 384], dt.float32, tag=f"po{j}",
                                          name=f"pso_{hci}_{blk}_{j}")
                        ps.append((pse, pso))
                    # center first (start=True, full width)
                    for j in range(2):
                        pse, pso = ps[j]
                        nc.tensor.matmul(pse[:, 0:hn], wtile[0:64, 0:128],
                                         xs[j][0:64, 0:hn], start=True,
                                         stop=False)
                        nc.tensor.matmul(pso[:, 0:hn], wtile[64:128, 0:128],
                                         xs[j][64:128, 0:hn], start=True,
                                         stop=False)
                    # slots narrow->wide; k=0 last (stop=True, full width)
                    for (_, k, col, gkn, wk) in by_chunk[hci]:
                        last = k == 0
                        wks = wtile[:, (k + 1) * 128:(k + 2) * 128]
                        gk = gts[k]
                        for j in range(2):
                            p = 2 * blk + j
                            pse, pso = ps[j]
                            nc.tensor.matmul(pse[:, 0:wk], wks[0:64, :],
                                             gk[0:64, p, 0:wk],
                                             start=False, stop=last)
                            nc.tensor.matmul(pso[:, 0:wk], wks[64:128, :],
                                             gk[64:128, p, 0:wk],
                                             start=False, stop=last)
                    # epilogue: multiply by inv (broadcast along partitions)
                    for j in range(2):
                        p = 2 * blk + j
                        pse, pso = ps[j]
                        oe = opool.tile([128, 384], dt.bfloat16, tag=f"oe{j}")
                        oo = opool.tile([128, 384], dt.bfloat16, tag=f"oo{j}")
                        nc.vector.tensor_mul(oe[:, 0:hv], pse[:, 0:hv],
                                             invt[:, off:off + hv])
                        nc.vector.tensor_mul(oo[:, 0:hv], pso[:, 0:hv],
                                             invt[:, off:off + hv])
                        nc.sync.dma_start(y[2 * p, :, off:off + hv], oe[:, 0:hv])
                        nc.sync.dma_start(y[2 * p + 1, :, off:off + hv],
                                          oo[:, 0:hv])
    nc.finalize()
    return nc


def _wrap_idx(idx_1d):
    """index list -> [128, n/16] int16 wrapped (pos i at partition i%16, slot i//16)."""
    n = idx_1d.shape[0]
    w = idx_1d.reshape(n // 16, 16).T
    return np.tile(w, (8, 1)).astype(np.int16)


def _segments(counts):
    """Per (chunk, slot) gather/matmul extents from valid-neighbor counts.

    Returns (segs, totw): segs = [(chunk, k, idx_col_off, gkn, wk)] in issue
    order (narrow slots first, slot 0 last per chunk); totw = total idx cols.
    """
    nk = [int((counts > k).sum()) for k in range(K)]
    segs = []
    col = 0
    for c, hn in enumerate(HCS):
        start = HC_OFF[c]
        for k in list(range(K - 1, 0, -1)) + [0]:
            if nk[k] <= start:
                continue
            wk = hn if k == 0 else min(nk[k] - start, hn)
            gkn = ((wk + 127) // 128) * 128
            segs.append((c, k, col, gkn, wk))
            col += gkn // 16
    return segs, col


def _host_prep(x, neighbors, weight_center, weight_neighbors, bias):
    x = np.asarray(x, np.float32)
    nb = np.asarray(neighbors)
    wc = np.asarray(weight_center, np.float32)
    wn = np.asarray(weight_neighbors, np.float32)

    mask = nb >= 0
    counts = mask.sum(1)
    perm = np.argsort(-counts, kind="stable")              # h sorted by count desc
    inv = (1.0 / (1.0 + counts[perm])).astype(np.float32)  # [H] permuted order
    invp = np.concatenate([inv, np.ones(Hp - H, np.float32)])
    inv_bcast = np.broadcast_to(invp, (128, Hp)).copy()

    # safe idx: rows in permuted order, values = ORIGINAL hex id (= token id)
    safe = np.where(mask, nb, H).astype(np.int16)[perm]    # [H, K]
    safe_p = np.concatenate([safe, np.full((Hp - H, K), H, np.int16)])

    segs, totw = _segments(counts)
    it = np.zeros((128, totw), np.int16)
    for (c, k, col, gkn, wk) in segs:
        lst = np.full(gkn, H, np.int16)
        lst[:wk] = safe_p[HC_OFF[c]:HC_OFF[c] + wk, k]
        it[:, col:col + gkn // 16] = _wrap_idx(lst)

    # weights: lhsT [128, 7*128] bf16, chunk c: rows 0-63 = W.T, 64-127 = W.T
    wt = np.zeros((128, 7 * 128), np.float32)
    wt[0:64, 0:128] = wc.T
    wt[64:128, 0:128] = wc.T
    for k in range(K):
        wt[0:64, (k + 1) * 128:(k + 2) * 128] = wn[:, :, k].T
        wt[64:128, (k + 1) * 128:(k + 2) * 128] = wn[:, :, k].T
    wt = wt.astype(BF16)

    xb = x.astype(BF16)                                    # [B, 64, H]
    in_maps = []
    for cid in range(NCORES):
        xs = xb[cid * BL:(cid + 1) * BL]                   # [32, 64, H]
        # token table: token t on partition t%128, rank t//128
        xtok = np.zeros((NTOK, BL * C_IN), BF16)
        xtok[:H] = xs.transpose(2, 0, 1).reshape(H, BL * C_IN)
        xt = (xtok.reshape(NRANK, 128, BL * C_IN)
              .transpose(1, 0, 2).reshape(128, NRANK * BL * C_IN))
        xcc = np.zeros((NPAIR, 128, Hp), BF16)
        xcc[:, 0:64, :H] = xs[0::2][:, :, perm]
        xcc[:, 64:128, :H] = xs[1::2][:, :, perm]
        in_maps.append({
            "xt": np.ascontiguousarray(xt),
            "xc": xcc,
            "wt": wt,
            "inv": inv_bcast,
            "idxt": it,
        })
    return in_maps, segs, totw, perm


def kernel(x, neighbors, weight_center, weight_neighbors, bias):
    global LAST_RESULT
    from concourse.bass_utils import run_bass_kernel_spmd

    in_maps, segs, totw, perm = _host_prep(x, neighbors, weight_center,
                                           weight_neighbors, bias)
    key = (tuple(segs), totw)
    if _CACHE.get("key") != key:
        _CACHE["nc"] = _build_program(segs, totw)
        _CACHE["key"] = key
    nc = _CACHE["nc"]
    res = run_bass_kernel_spmd(nc, in_maps, core_ids=list(range(NCORES)),
                               trace=TRACE)
    LAST_RESULT = res
    out = np.concatenate([r["y"] for r in res.results], axis=0).astype(np.float32)
    inv_perm = np.empty_like(perm)
    inv_perm[perm] = np.arange(perm.shape[0])
    out = out[:, :, inv_perm]                   # undo count-sort of h
    b = np.asarray(bias, np.float32)
    if np.any(b != 0.0):
        # reference adds bias after the divide; device epilogue skips it
        out = out + b[None, :, None]
    return np.ascontiguousarray(out)


# revision 9
# speedup vs baseline: 1.7839x; 1.7839x over previous
"""ConvHex (hex-grid graph conv) Trainium2 Bass kernel.

out[b,o,h] = (Wc@x[b,:,h] + sum_k Wn[:,:,k]@x[b,:,nb[h,k]]*mask) / (1+#valid) + bias

Strategy (8 NeuronCores, data-parallel over batch B=256 -> 32/core):
- x in HBM as a token table xt [1040, 2048] bf16: token t (= hex id,
  1039 = zero pad) is the 4KB row x[0:32, 0:64, t].
- Neighbor gather: HBM-source dma_gather(transpose=True), 4KB tokens
  (one descriptor = one hex for all 32 batches). (SBUF-source gathers
  measured ~7x slower per descriptor: each token reads one partition =
  one SBUF AXI port; don't go back.)
- h count-sorted (desc valid-neighbor count) so slot k is active only for
  the first nk[k] columns; gathers and matmul widths trimmed per slot.
- Matmul: center (start=True, full width) then slots narrow->wide, slot 0
  last (stop=True, full width). K=64 contraction, batch pairs: even batch
  on PE rows 0-63 -> psum_e, odd on 64-127 -> psum_o.
- Epilogue: DVE multiply by 1/(1+count) broadcast, bf16 out. bias added
  on host only if nonzero (zero in this problem).
"""
import os
import numpy as np
import ml_dtypes

B, C_IN, C_OUT, H, K = 256, 64, 128, 1039, 6
NCORES = 8
BL = B // NCORES            # 32 batches per core
NPAIR = BL // 2             # 16
Hp = H + 1                  # 1040; token/column H (=1039) is the zero pad
HCS = [384, 384, 272]       # h-chunks (psum bank sized)
HC_OFF = [0, 384, 768]
BF16 = ml_dtypes.bfloat16

TRACE = bool(int(os.environ.get("KERNEL_TRACE", "0")))
LAST_RESULT = None

_CACHE = {}


def _build_program(segs, totw):
    import concourse.mybir as mybir
    import concourse.tile as tile
    from concourse import bacc

    nc = bacc.Bacc(name="convhex")
    dt = mybir.dt
    xt_d = nc.dram_tensor("xt", [Hp, BL * C_IN], dt.bfloat16,
                          kind="ExternalInput")
    xc_d = nc.dram_tensor("xc", [NPAIR, 128, Hp], dt.bfloat16,
                          kind="ExternalInput")
    wt_d = nc.dram_tensor("wt", [128, 7 * 128], dt.bfloat16,
                          kind="ExternalInput")
    inv_d = nc.dram_tensor("inv", [128, Hp], dt.float32, kind="ExternalInput")
    it_d = nc.dram_tensor("idxt", [128, totw], dt.int16, kind="ExternalInput")
    y = nc.dram_tensor("y", [BL, 128, H], dt.bfloat16, kind="ExternalOutput")

    by_chunk = [[s for s in segs if s[0] == c] for c in range(len(HCS))]

    with tile.TileContext(nc) as tc:
        with tc.tile_pool(name="const", bufs=1) as cpool, \
             tc.tile_pool(name="gat", bufs=9) as gpool, \
             tc.tile_pool(name="xcp", bufs=8) as xcpool, \
             tc.tile_pool(name="osb", bufs=2) as opool, \
             tc.tile_pool(name="ps", bufs=2, space="PSUM") as pspool:
            wtile = cpool.tile([128, 7 * 128], dt.bfloat16)
            nc.sync.dma_start(wtile[:], wt_d[:, :])
            invt = cpool.tile([128, Hp], dt.float32)
            nc.sync.dma_start(invt[:], inv_d[:, :])
            itt = cpool.tile([128, totw], dt.int16)
            nc.sync.dma_start(itt[:], it_d[:, :])

            for hci, hn in enumerate(HCS):
                off = HC_OFF[hci]
                hv = min(hn, H - off)   # valid output columns
                gts = {}
                for (_, k, col, gkn, wk) in by_chunk[hci]:
                    gt = gpool.tile([128, NPAIR, gkn], dt.bfloat16,
                                    tag="g", name=f"g_{hci}_{k}")
                    nc.gpsimd.dma_gather(
                        gt[:], xt_d[:, :],
                        itt[:, col:col + gkn // 16],
                        num_idxs=gkn, num_idxs_reg=gkn,
                        elem_size=BL * C_IN,
                        elem_step=BL * C_IN,
                        transpose=True,
                    )
                    gts[k] = gt
                for blk in range(NPAIR // 2):
                    ps = []
                    xs = []
                    for j in range(2):
                        p = 2 * blk + j
                        xct = xcpool.tile([128, 384], dt.bfloat16, tag="xc")
                        nc.sync.dma_start(xct[:, 0:hn], xc_d[p, :, off:off + hn])
                        xs.append(xct)
                        pse = pspool.tile([128, 384], dt.float32, tag=f"pe{j}",
                                          name=f"pse_{hci}_{blk}_{j}")
                        pso = pspool.tile([128, 384], dt.float32, tag=f"po{j}",
                                          name=f"pso_{hci}_{blk}_{j}")
                        ps.append((pse, pso))
                    # center first (start=True, full width)
                    for j in range(2):
                        pse, pso = ps[j]
                        nc.tensor.matmul(pse[:, 0:hn], wtile[0:64, 0:128],
                                         xs[j][0:64, 0:hn], start=True,
                                         stop=False)
                        nc.tensor.matmul(pso[:, 0:hn], wtile[64:128, 0:128],
                                         xs[j][64:128, 0:hn], start=True,
                                         stop=False)
                    # slots narrow->wide; k=0 last (stop=True, full width)
                    for (_, k, col, gkn, wk) in by_chunk[hci]:
                        last = k == 0
                        wks = wtile[:, (k + 1) * 128:(k + 2) * 128]
                        gk = gts[k]
                        for j in range(2):
                            p = 2 * blk + j
                            pse, pso = ps[j]
                            nc.tensor.matmul(pse[:, 0:wk], wks[0:64, :],
                                             gk[0:64, p, 0:wk],
                                             start=False, stop=last)
                            nc.tensor.matmul(pso[:, 0:wk], wks[64:128, :],
                                             gk[64:128, p, 0:wk],
                                             start=False, stop=last)
                    # epilogue: multiply by inv (broadcast along partitions)
                    for j in range(2):
                        p = 2 * blk + j
                        pse, pso = ps[j]
                        oe = opool.tile([128, 384], dt.bfloat16, tag=f"oe{j}")
                        oo = opool.tile([128, 384], dt.bfloat16, tag=f"oo{j}")
                        nc.vector.tensor_mul(oe[:, 0:hv], pse[:, 0:hv],
                                             invt[:, off:off + hv])
                        nc.vector.tensor_mul(oo[:, 0:hv], pso[:, 0:hv],
                                             invt[:, off:off + hv])
                        nc.scalar.dma_start(y[2 * p, :, off:off + hv],
                                            oe[:, 0:hv])
                        nc.scalar.dma_start(y[2 * p + 1, :, off:off + hv],
                                            oo[:, 0:hv])
    nc.finalize()
    return nc


def _wrap_idx(idx_1d):
    """index list -> [128, n/16] int16 wrapped (pos i at partition i%16, slot i//16)."""
    n = idx_1d.shape[0]
    w = idx_1d.reshape(n // 16, 16).T
    return np.tile(w, (8, 1)).astype(np.int16)


def _segments(counts):
    """Per (chunk, slot) gather/matmul extents from valid-neighbor counts.

    Returns (segs, totw): segs = [(chunk, k, idx_col_off, gkn, wk)] in issue
    order (narrow slots first, slot 0 last per chunk); totw = total idx cols.
    """
    nk = [int((counts > k).sum()) for k in range(K)]
    segs = []
    col = 0
    for c, hn in enumerate(HCS):
        start = HC_OFF[c]
        for k in list(range(K - 1, 0, -1)) + [0]:
            if nk[k] <= start:
                continue
            wk = hn if k == 0 else min(nk[k] - start, hn)
            gkn = ((wk + 127) // 128) * 128
            segs.append((c, k, col, gkn, wk))
            col += gkn // 16
    return segs, col


def _host_prep(x, neighbors, weight_center, weight_neighbors, bias):
    x = np.asarray(x, np.float32)
    nb = np.asarray(neighbors)
    wc = np.asarray(weight_center, np.float32)
    wn = np.asarray(weight_neighbors, np.float32)

    mask = nb >= 0
    counts = mask.sum(1)
    perm = np.argsort(-counts, kind="stable")              # h sorted by count desc
    inv = (1.0 / (1.0 + counts[perm])).astype(np.float32)  # [H] permuted order
    invp = np.concatenate([inv, np.ones(Hp - H, np.float32)])
    inv_bcast = np.broadcast_to(invp, (128, Hp)).copy()

    # safe idx: rows in permuted order, values = ORIGINAL hex id (= token id)
    safe = np.where(mask, nb, H).astype(np.int16)[perm]    # [H, K]
    safe_p = np.concatenate([safe, np.full((Hp - H, K), H, np.int16)])

    segs, totw = _segments(counts)
    it = np.zeros((128, totw), np.int16)
    for (c, k, col, gkn, wk) in segs:
        lst = np.full(gkn, H, np.int16)
        lst[:wk] = safe_p[HC_OFF[c]:HC_OFF[c] + wk, k]
        it[:, col:col + gkn // 16] = _wrap_idx(lst)

    # weights: lhsT [128, 7*128] bf16, chunk c: rows 0-63 = W.T, 64-127 = W.T
    wt = np.zeros((128, 7 * 128), np.float32)
    wt[0:64, 0:128] = wc.T
    wt[64:128, 0:128] = wc.T
    for k in range(K):
        wt[0:64, (k + 1) * 128:(k + 2) * 128] = wn[:, :, k].T
        wt[64:128, (k + 1) * 128:(k + 2) * 128] = wn[:, :, k].T
    wt = wt.astype(BF16)

    xb = x.astype(BF16)                                    # [B, 64, H]
    in_maps = []
    for cid in range(NCORES):
        xs = xb[cid * BL:(cid + 1) * BL]                   # [32, 64, H]
        # token table: row t = x[:, :, t] flattened (b, c); row 1039 zeros
        xt = np.zeros((Hp, BL * C_IN), BF16)
        xt[:H] = xs.transpose(2, 0, 1).reshape(H, BL * C_IN)
        xcc = np.zeros((NPAIR, 128, Hp), BF16)
        xcc[:, 0:64, :H] = xs[0::2][:, :, perm]
        xcc[:, 64:128, :H] = xs[1::2][:, :, perm]
        in_maps.append({
            "xt": np.ascontiguousarray(xt),
            "xc": xcc,
            "wt": wt,
            "inv": inv_bcast,
            "idxt": it,
        })
    return in_maps, segs, totw, perm


def kernel(x, neighbors, weight_center, weight_neighbors, bias):
    global LAST_RESULT
    from concourse.bass_utils import run_bass_kernel_spmd

    in_maps, segs, totw, perm = _host_prep(x, neighbors, weight_center,
                                           weight_neighbors, bias)
    key = (tuple(segs), totw)
    if _CACHE.get("key") != key:
        _CACHE["nc"] = _build_program(segs, totw)
        _CACHE["key"] = key
    nc = _CACHE["nc"]
    res = run_bass_kernel_spmd(nc, in_maps, core_ids=list(range(NCORES)),
                               trace=TRACE)
    LAST_RESULT = res
    out = np.concatenate([r["y"] for r in res.results], axis=0).astype(np.float32)
    inv_perm = np.empty_like(perm)
    inv_perm[perm] = np.arange(perm.shape[0])
    out = out[:, :, inv_perm]                   # undo count-sort of h
    b = np.asarray(bias, np.float32)
    if np.any(b != 0.0):
        # reference adds bias after the divide; device epilogue skips it
        out = out + b[None, :, None]
    return np.ascontiguousarray(out)
